# revision 27
# baseline (speedup 1.0000x reference)
"""Trainium2 Bass kernel: multi-head attention with per-head QK LayerNorm.

Problem shapes: B=2, S=2048, D=1024, H=16 heads, head_dim=64, fp32 in/out.

Sharding (8 cores): core c handles batch b = c//4 and head-group g = c%4
(4 heads = 256 qkv dims). Each core computes its heads' attention and a
partial out-projection; the host sums the 4 partials per batch entry
(tensor-parallel all-reduce done on host at unshard time) and adds o_b.

Key algebraic restructurings (all exact, modulo fp rounding):
  - LN mean subtraction and gain g are linear => folded into q_w/k_w (and
    biases) on the host.  Kernel computes qg = g*(q - mean(q)) directly.
  - LN variance = sum(w_d * qg_d^2) with w_d = 1/(64*g_d^2): computed on
    device from qg^2 via small block-diagonal stats matmuls.
  - rstd_q is folded into qT columns and tau*rstd_k into kT columns
    (via partition-broadcast DMAs), so softmax is a bare exp() of the
    raw scores.  Scores are computed TRANSPOSED: [kv on partitions,
    q on free], which feeds AV directly with no PE transposes.
  - softmax max-subtraction is skipped: post-LN rows have norm 8, so
    |scores| <= 8 and exp() stays in range.
  - sum(exp) over kv falls out of the AV matmul via a ones-column
    appended to V.  Normalization happens on attT eviction.

Perf notes (v2, fp16 + software-pipelined emission):
  - All matmul operands fp16, all matmuls N=512.  fp16 streams at the
    full 1 col/cycle PE rate and enables FWL weight loads; 11 mantissa
    bits keep final rel err ~1e-3 (bf16 would be marginal).
  - Phase 2 is ACT(exp)-bound (128 x 1147ns merged exps).  Engine
    queues execute in order, so emission is software-pipelined:
    QK(j+1) is emitted BEFORE exp(j)/AV(j) so the PE never sits behind
    an exp-dependent AV when the next scores could be computing.
  - QK pairs go to row tiles (0,0)/(64,0) (lhsT partitions 0-63/64-127)
    and run CONCURRENTLY on the PE (measured 109ns each @N=512 warm).
  - The c1 projection chains, v is upfront, out-projections and the
    remaining q chains are WOVEN into the exp stream as PE filler --
    this both hides phase-1 latency and keeps PE busy% high enough
    that the HAM clock gate stays at 2.4 GHz.
  - Projection chains are split A (8 proj mms + evict + square) /
    B (stats mm + sqrt + recip + bcast + scale) and B is emitted >=2
    exp-periods after A so the PE queue never stalls on GpSimd square.
  - PSUM: scores 2x[128,2,512] (4 banks) + AV accum 2 + acc pool
    (proj/stats/out-proj) 2 = 8 banks exactly.
"""

import os
import sys

import numpy as np

for _p in ("/opt/trn_rl_repo",):
    if _p not in sys.path:
        sys.path.append(_p)

# ---- problem constants (hardcoded; kernel.py must be self-contained) ----
B, S, D, H, HD = 2, 2048, 1024, 16, 64
EPS = 1e-5
NCORES = 8
GPC = 4            # cores per batch entry (head-groups)
HL = H // GPC      # 4 local heads
DL = HL * HD       # 256 local qkv dims
P = 128
KC = D // P        # 8 contraction chunks for projections
CL = DL // P       # 2 local-dim partition chunks (head pairs)
SB = 512           # free-dim block (= one PSUM bank of fp32)
NSB = S // SB      # 4 blocks
NKV = S // P       # 16 kv chunks
STW = 33           # stats lhsT cols: head vars at partitions 0 and 32

_CACHE = {}


def _build_nc():
    """Build the (single, SPMD-shared) Bass program for one core."""
    import concourse.bass as bass
    import concourse.mybir as mybir
    import concourse.tile as tile
    from concourse import bacc
    from concourse.dve_ops import RECIPROCAL_APPROX_FAST, RECIP_APPROX_FAST_CONSTS

    f32 = mybir.dt.float32
    f16 = mybir.dt.float16
    AF = mybir.ActivationFunctionType
    rc = RECIP_APPROX_FAST_CONSTS

    def recip(nc, out, in_):
        # ~51-ULP reciprocal in a single DVE pass (vs ~6 cyc/elem exact).
        return nc.vector._custom_dve(
            RECIPROCAL_APPROX_FAST, out=out, in0=in_,
            s0=rc["s0"], s1=rc["s1"], imm2=rc["imm2"],
        )

    nc = bacc.Bacc(trn_type="TRN2")

    xT_d = nc.dram_tensor("xT", [KC, P, S], f16, kind="ExternalInput")
    wqT_d = nc.dram_tensor("wqT", [KC, P, DL], f16, kind="ExternalInput")
    wkT_d = nc.dram_tensor("wkT", [KC, P, DL], f16, kind="ExternalInput")
    wvT_d = nc.dram_tensor("wvT", [KC, P, DL], f16, kind="ExternalInput")
    woT_d = nc.dram_tensor("woT", [CL, P, D], f16, kind="ExternalInput")
    qb_d = nc.dram_tensor("qb", [CL, P, 1], f32, kind="ExternalInput")
    kb_d = nc.dram_tensor("kb", [CL, P, 1], f32, kind="ExternalInput")
    vb_d = nc.dram_tensor("vb", [1, DL], f32, kind="ExternalInput")
    wsq_d = nc.dram_tensor("wsq", [CL, P, STW], f16, kind="ExternalInput")
    wsk_d = nc.dram_tensor("wsk", [CL, P, STW], f16, kind="ExternalInput")
    out_d = nc.dram_tensor("out", [NKV, P, D], f16, kind="ExternalOutput")

    with tile.TileContext(nc) as tc:
        with tc.tile_pool(name="big", bufs=1) as big:
            # ---- persistent SBUF; DMA issue order = need order ----
            xt = [big.tile([P, S], f16, name=f"xt{k}") for k in range(KC)]
            wk_sb = [big.tile([P, DL], f16, name=f"wk{k}") for k in range(KC)]
            wq_sb = [big.tile([P, DL], f16, name=f"wq{k}") for k in range(KC)]
            wv_sb = [big.tile([P, DL], f16, name=f"wv{k}") for k in range(KC)]
            # xt arrives in per-s-block quarters, sb0 first, so the first
            # projection chain starts after ~1MB instead of the full 4MB.
            for k in range(KC):
                nc.sync.dma_start(xt[k][:, 0:SB], xT_d[k, :, 0:SB])
                nc.sync.dma_start(wk_sb[k], wkT_d[k])
            kb_sb = big.tile([P, CL, 1], f32, name="kb_sb")
            qb_sb = big.tile([P, CL, 1], f32, name="qb_sb")
            wsq_sb = big.tile([P, CL, STW], f16, name="wsq_sb")
            wsk_sb = big.tile([P, CL, STW], f16, name="wsk_sb")
            for c in range(CL):
                nc.sync.dma_start(kb_sb[:, c, :], kb_d[c])
                nc.sync.dma_start(qb_sb[:, c, :], qb_d[c])
                nc.sync.dma_start(wsq_sb[:, c, :], wsq_d[c])
                nc.sync.dma_start(wsk_sb[:, c, :], wsk_d[c])
            for sb in range(1, NSB):
                for k in range(KC):
                    nc.sync.dma_start(xt[k][:, sb * SB:(sb + 1) * SB],
                                      xT_d[k, :, sb * SB:(sb + 1) * SB])
            for k in range(KC):
                nc.sync.dma_start(wq_sb[k], wqT_d[k])
            for k in range(KC):
                nc.sync.dma_start(wv_sb[k], wvT_d[k])
            vb_bc = big.tile([P, DL], f32, name="vb_bc")
            nc.sync.dma_start(vb_bc, vb_d[:].to_broadcast((P, DL)))
            wo_sb = big.tile([P, CL, D], f16, name="wo_sb")
            for c in range(CL):
                nc.sync.dma_start(wo_sb[:, c, :], woT_d[c])

            kT_sb = big.tile([P, CL, S], f16, name="kT_sb")
            qTs_sb = big.tile([P, CL, S], f16, name="qTs_sb")
            vaug_sb = big.tile([P, NKV, HL, HD + 1], f16, name="vaug_sb")
            attT_sb = big.tile([P, CL, S], f16, name="attT_sb")
            nc.vector.memset(vaug_sb[:, :, :, HD:HD + 1], 1.0)
            # onesel broadcasts rstd rows (partitions 0 and 32) to the 128
            # qkv partitions via a matmul: col m reads partition 0 (m<64)
            # or partition 32 (m>=64).
            onesel = big.tile([STW, P], f16, name="onesel")
            nc.vector.memset(onesel, 0.0)
            nc.vector.memset(onesel[0:1, 0:HD], 1.0)
            nc.vector.memset(onesel[32:33, HD:P], 1.0)

            with tc.tile_pool(name="acc", bufs=2, space="PSUM") as acc, \
                 tc.tile_pool(name="qk", bufs=2, space="PSUM") as qk, \
                 tc.tile_pool(name="av", bufs=1, space="PSUM") as avp, \
                 tc.tile_pool(name="sq", bufs=3) as sq, \
                 tc.tile_pool(name="ev", bufs=4) as ev, \
                 tc.tile_pool(name="ex", bufs=4) as exp_pool:

                SIDES = {
                    "k": (wk_sb, kb_sb, wsk_sb, kT_sb, 64.0),
                    "q": (wq_sb, qb_sb, wsq_sb, qTs_sb, 1.0),
                }
                i32 = mybir.dt.int32
                ALU = mybir.AluOpType
                RSQRT_MAGIC = 0x5F3759DF

                def dve_rsqrt(z, rr_out):
                    """rr_out(f16) = z**-0.5 via quake seed + 2 Newton
                    iterations, entirely on the Vector engine (no ACT table,
                    no broken partition-broadcast)."""
                    sh = ev.tile([STW, SB], i32, name="sh", bufs=2)
                    nc.vector.tensor_scalar(
                        sh, z.bitcast(i32), 1, None,
                        op0=ALU.logical_shift_right)
                    y0i = ev.tile([STW, SB], i32, name="y0i", bufs=2)
                    nc.vector.tensor_scalar(
                        y0i, sh, -1, RSQRT_MAGIC,
                        op0=ALU.mult, op1=ALU.add)
                    y = y0i.bitcast(f32)
                    for it in range(2):
                        t = ev.tile([STW, SB], f32, name="t", tag="t", bufs=4)
                        nc.vector.tensor_mul(t, z, y)
                        t2 = ev.tile([STW, SB], f32, name="t2", tag="t2",
                                     bufs=4)
                        nc.vector.tensor_mul(t2, t, y)
                        w = ev.tile([STW, SB], f32, name="w", tag="w", bufs=4)
                        nc.vector.tensor_scalar(
                            w, t2, -0.5, 1.5, op0=ALU.mult, op1=ALU.add)
                        if it == 0:
                            y1 = ev.tile([STW, SB], f32, name="y1", bufs=2)
                            nc.vector.tensor_mul(y1, y, w)
                            y = y1
                        else:
                            nc.vector.tensor_mul(rr_out, y, w)

                def chain_items(side, c, sb):
                    """q/k projection chain, split A/B1/B2 so the PE queue
                    never waits on the GpSimd square (A->B1) or the DVE
                    rsqrt latency (B1->B2)."""
                    wlist, bcol, wst, dst, scv = SIDES[side]
                    st = {}

                    def part_a():
                        ph = acc.tile([P, SB], f32, name="ph", tag="acc")
                        for k in range(KC):
                            nc.tensor.matmul(
                                ph, wlist[k][:, c * P:(c + 1) * P],
                                xt[k][:, sb * SB:(sb + 1) * SB],
                                start=(k == 0), stop=(k == KC - 1),
                            )
                        tr = sq.tile([P, SB], f16, name="tr_t")
                        nc.vector.tensor_scalar_add(tr, ph, bcol[:, c, :])
                        # DVE, not GpSimd: mixing tensor ops with
                        # partition_broadcast on GpSimd ping-pongs its ucode
                        # library (~3-6us hidden LIBRARY_RELOAD per switch).
                        qsq = sq.tile([P, SB], f16, name="sq_t")
                        nc.vector.tensor_mul(qsq, tr, tr)
                        st["tr"], st["qsq"] = tr, qsq

                    def part_b1():
                        # stats lhsT has 33 cols: head0 var -> partition 0,
                        # head1 var -> partition 32 (engines may only access
                        # partition bases aligned to 32).
                        stp = acc.tile([STW, SB], f32, name="stp", tag="acc")
                        nc.tensor.matmul(stp, wst[:, c, :], st["qsq"],
                                         start=True, stop=True)
                        z = ev.tile([STW, SB], f32, name="z", bufs=2)
                        nc.vector.tensor_scalar(
                            z, stp, scv, scv * EPS,
                            op0=ALU.mult, op1=ALU.add)
                        rr = ev.tile([STW, SB], f16, name="rr", bufs=2)
                        dve_rsqrt(z, rr)
                        st["rr"] = rr

                    def part_b2():
                        # broadcast rstd rows to all 128 partitions on the PE
                        # (partition_broadcast with out base 64 is broken on
                        # HW; SBUF->SBUF broadcast DMA has multi-us latency).
                        qsc = acc.tile([P, SB], f32, name="qsc", tag="acc")
                        nc.tensor.matmul(qsc, onesel, st["rr"],
                                         start=True, stop=True)
                        nc.vector.tensor_mul(
                            dst[:, c, sb * SB:(sb + 1) * SB], st["tr"], qsc)

                    return [("chain", part_a), ("chain", part_b1),
                            ("chain", part_b2)]

                def v_item(mc):
                    def f():
                        pv = acc.tile([P, SB], f32, name="pv",
                                      tag="acc")[:, :DL]
                        for k in range(KC):
                            nc.tensor.matmul(
                                pv, xt[k][:, mc * P:(mc + 1) * P], wv_sb[k],
                                start=(k == 0), stop=(k == KC - 1),
                            )
                        nc.vector.tensor_add(
                            vaug_sb[:, mc, :, 0:HD],
                            pv.rearrange("p (h d) -> p h d", d=HD),
                            vb_bc.rearrange("p (h d) -> p h d", d=HD),
                        )
                    return [("v", f)]

                def op_item(m, nb, use_qk=False):
                    def f():
                        if use_qk:
                            pon = qk.tile([P, 2, SB], f32,
                                          name="qk_t")[:, 0, :]
                        else:
                            pon = acc.tile([P, SB], f32, name="pon",
                                           tag="acc")
                        for c in range(CL):
                            nc.tensor.matmul(
                                pon, attT_sb[:, c, m * P:(m + 1) * P],
                                wo_sb[:, c, nb * SB:(nb + 1) * SB],
                                start=(c == 0), stop=(c == CL - 1),
                            )
                        osb = ev.tile([P, SB], f16, name="osb")
                        nc.vector.tensor_copy(osb, pon)
                        nc.sync.dma_start(
                            out_d[m, :, nb * SB:(nb + 1) * SB], osb)
                    return [("op", f)]

                # ---- upfront: k(c0) x4, q(c0,sb0), v0-v1 only; the rest
                # of v and all other chains weave into the exp stream.
                # A/B1/B2 staged so the PE never waits on the GpSimd square
                # (A->B1) or the DVE rsqrt (B1->B2).
                # q(c0,sb0) emitted right after k(c0,sb0): its rsqrt is
                # 2nd in the in-order DVE queue, so qTs(c0,sb0) -- which
                # gates the FIRST exp -- is ready ~30us earlier than if all
                # four k-chain rsqrts queued ahead of it.
                ch = [chain_items("k", 0, 0), chain_items("q", 0, 0),
                      chain_items("k", 0, 1), chain_items("k", 0, 2),
                      chain_items("k", 0, 3)]
                A = [c[0][1] for c in ch]
                B1 = [c[1][1] for c in ch]
                B2 = [c[2][1] for c in ch]
                for fn in (A[0], A[1], B1[0], A[2], B1[1], B2[0],
                           v_item(0)[0][1], A[3], B2[1], v_item(1)[0][1],
                           A[4], B1[2], B1[3], B2[2], B1[4], B2[3], B2[4]):
                    fn()

                # ---- filler schedule: block idx -> list of (kind, fn) ----
                fillers = {i: [] for i in range(8)}
                PAD = ("pad", lambda: None)
                # block order is c0-major: blocks 0-3 = (qb0..3, c0),
                # blocks 4-7 = (qb0..3, c1).  Every chain's B2 must be
                # emitted before the first QK of the block that consumes its
                # kT/qTs slice (emission order IS the dependency order).
                vs = [v_item(mc)[0] for mc in range(2, NKV)]
                q01 = chain_items("q", 0, 1)
                # block0: v chunks just-in-time (v(mc) >=2 periods before its
                # AV) with q(c0,sb1) finishing by slot 14.
                fillers[0] = (vs[0:8] + [q01[0]] + vs[8:10] + [q01[1]] +
                              vs[10:12] + [q01[2]] + vs[12:14])

                def two_chains(ca, cb):
                    return [ca[0], cb[0], ca[1], cb[1], PAD, ca[2], cb[2]]

                fillers[1] = two_chains(chain_items("q", 0, 2),
                                        chain_items("k", 1, 0))
                q03, k11, q10 = (chain_items("q", 0, 3),
                                 chain_items("k", 1, 1),
                                 chain_items("q", 1, 0))
                fillers[2] = [q03[0], k11[0], q03[1], q10[0], k11[1], PAD,
                              q03[2], q10[1], k11[2], PAD, q10[2]]
                fillers[3] = two_chains(chain_items("k", 1, 2),
                                        chain_items("k", 1, 3))

                def one_chain(c):
                    return [c[0], PAD, c[1], PAD, PAD, c[2]]

                fillers[4] = one_chain(chain_items("q", 1, 1))
                fillers[5] = one_chain(chain_items("q", 1, 2))
                fillers[6] = one_chain(chain_items("q", 1, 3))
                # out-projections: op(qb) needs attT(qb,c0) [block qb] and
                # attT(qb,c1) [block 4+qb]
                opi = {qb: [op_item(m, nb)[0]
                            for m in range(qb * 4, qb * 4 + 4)
                            for nb in range(D // SB)]
                       for qb in range(NSB - 1)}
                fillers[5] += opi[0]
                fillers[6] += opi[1]
                fillers[7] = opi[2]
                tail_ops = [op_item(m, nb, use_qk=(m + nb) % 2 == 1)[0]
                            for m in range(12, 16)
                            for nb in range(D // SB)]

                # ---- phase 2: software-pipelined attention stream ----
                blocks = [(qb, c) for c in range(CL) for qb in range(NSB)]
                groups = [(bi, qb, c, j)
                          for bi, (qb, c) in enumerate(blocks)
                          for j in range(NKV)]
                sc_of = {}
                avs_of = {}

                def emit_qk(g):
                    bi, qb, c, j = groups[g]
                    sc2 = qk.tile([P, 2, SB], f32, name="qk_t")
                    q0 = qb * SB
                    for h in range(2):
                        po = h * HD
                        nc.tensor.matmul(
                            sc2[:, h, :],
                            kT_sb[po:po + HD, c, j * P:(j + 1) * P],
                            qTs_sb[po:po + HD, c, q0:q0 + SB],
                            start=True, stop=True,
                        )
                    sc_of[g] = sc2

                AVLAG = 3  # AV trails exp by 3 groups: absorbs the norm
                #            latency of the previous block (av bufs=1) without
                #            blocking the in-order PE queue / starving ACT.
                ex_of = {}

                def emit_exp(g):
                    sc2 = sc_of.pop(g)
                    ex2 = exp_pool.tile([P, 2, SB], f16, name="ex_t")
                    nc.scalar.activation(ex2, sc2, AF.Exp)
                    ex_of[g] = ex2

                def emit_av(g):
                    bi, qb, c, j = groups[g]
                    ex2 = ex_of.pop(g)
                    if j == 0:
                        avs_of[bi] = [
                            avp.tile([HD + 1, SB], f32, name=f"av{h}",
                                     tag=f"av{h}") for h in range(2)]
                    for h in range(2):
                        nc.tensor.matmul(
                            avs_of[bi][h],
                            vaug_sb[:, j, c * 2 + h, :],
                            ex2[:, h, :],
                            start=(j == 0), stop=(j == NKV - 1),
                        )
                    if j == NKV - 1:
                        avs = avs_of.pop(bi)
                        q0 = qb * SB
                        for h in range(2):
                            po = h * HD
                            # plain copy handles the partition shift (64->0);
                            # partition-shifted custom-DVE ops are not
                            # trustworthy on HW.
                            srow = ev.tile([1, SB], f32, name="srow")
                            nc.vector.tensor_copy(srow, avs[h][HD:HD + 1, :])
                            rrow = ev.tile([1, SB], f32, name="rrow")
                            recip(nc, rrow, srow)
                            rbc = ev.tile([HD, SB], f32, name="rbc")
                            nc.gpsimd.partition_broadcast(
                                rbc, rrow[0:1, :], HD)
                            nc.vector.tensor_mul(
                                attT_sb[po:po + HD, c, q0:q0 + SB],
                                avs[h][0:HD, :], rbc)

                emit_qk(0)
                NG = len(groups)
                for g in range(NG + AVLAG):
                    if g + 1 < NG:
                        emit_qk(g + 1)
                    if g < NG:
                        emit_exp(g)
                    if g - AVLAG >= 0:
                        emit_av(g - AVLAG)
                    if g < NG:
                        bi, qb, c, j = groups[g]
                        # filler items per kv chunk (ops only once attT of
                        # the previous qb has had time to normalize); pop 2
                        # when the remaining slots would not drain the list
                        fl = fillers[bi]
                        npop = 1
                        if len(fl) > NKV - j:
                            npop = 2
                        minj = {"op": 4, "chain": 3}
                        for _ in range(npop):
                            if fl and j >= minj.get(fl[0][0], 0):
                                fl.pop(0)[1]()

                # tail: out-projection of the last q-block
                for it in tail_ops:
                    it[1]()

    nc.compile()
    return nc


def _prepare_core_inputs(inputs):
    """Fold LN centering/gain into weights; shard per core; cast fp16."""
    q = np.asarray(inputs["query"], np.float32)
    q_w = np.asarray(inputs["q_w"], np.float64)
    k_w = np.asarray(inputs["k_w"], np.float64)
    v_w = np.asarray(inputs["v_w"], np.float32)
    o_w = np.asarray(inputs["o_w"], np.float32)
    q_b = np.asarray(inputs["q_b"], np.float64)
    k_b = np.asarray(inputs["k_b"], np.float64)
    v_b = np.asarray(inputs["v_b"], np.float32)
    q_g = np.asarray(inputs["q_ln_g"], np.float64)
    k_g = np.asarray(inputs["k_ln_g"], np.float64)

    def fold(w, b, g):
        # per head block (64 out-dims): center across the block, scale by g
        w = w.reshape(H, HD, D)
        w = (w - w.mean(axis=1, keepdims=True)) * g[None, :, None]
        b = b.reshape(H, HD)
        b = (b - b.mean(axis=1, keepdims=True)) * g[None, :]
        return w.reshape(D, D), b.reshape(D).astype(np.float32)

    wq_f, qb_f = fold(q_w, q_b, q_g)
    wk_f, kb_f = fold(k_w, k_b, k_g)

    def stat_w(g):
        # w_dd = 1/(64*g_d^2), laid out [CL, P, 33] block-diagonal per c-half
        # (head0 -> col 0, head1 -> col 32: partition-32-aligned outputs)
        w = np.zeros((CL, P, STW), np.float64)
        for c in range(CL):
            for h in range(2):
                w[c, h * HD:(h + 1) * HD, 32 * h] = 1.0 / (HD * g[:HD] ** 2)
        return w.astype(np.float16)

    wsq = stat_w(np.asarray(inputs["q_ln_g"], np.float64))
    wsk = stat_w(np.asarray(inputs["k_ln_g"], np.float64))

    in_maps = []
    for c in range(NCORES):
        b, g = divmod(c, GPC)
        rows = slice(g * DL, (g + 1) * DL)
        in_maps.append({
            "xT": np.ascontiguousarray(q[b].T).reshape(KC, P, S).astype(np.float16),
            "wqT": np.ascontiguousarray(wq_f[rows].T).reshape(KC, P, DL).astype(np.float16),
            "wkT": np.ascontiguousarray(wk_f[rows].T).reshape(KC, P, DL).astype(np.float16),
            "wvT": np.ascontiguousarray(v_w[rows].T).reshape(KC, P, DL).astype(np.float16),
            "woT": np.ascontiguousarray(o_w[:, rows].T).reshape(CL, P, D).astype(np.float16),
            "qb": np.ascontiguousarray(qb_f[rows]).reshape(CL, P, 1),
            "kb": np.ascontiguousarray(kb_f[rows]).reshape(CL, P, 1),
            "vb": np.ascontiguousarray(v_b[rows]).reshape(1, DL),
            "wsq": wsq,
            "wsk": wsk,
        })
    return in_maps


def _install_ntff_shim():
    """The agent image's antenv lacks axon_hooks; recreate it so
    run_bass_kernel_spmd(trace=True) can capture NTFF profiles."""
    import types

    try:
        import antenv.axon_hooks  # noqa: F401
        return
    except ImportError:
        pass
    import antenv
    mod = types.ModuleType("antenv.axon_hooks")
    mod._hook = None
    mod.set_axon_ntff_profile_hook = lambda h: setattr(mod, "_hook", h)
    mod.get_axon_ntff_profile_hook = lambda: mod._hook
    sys.modules["antenv.axon_hooks"] = mod
    antenv.axon_hooks = mod
    try:
        from trn_agent_boot.trn_boot import _ntff_profile_via_ctypes
        hook = _ntff_profile_via_ctypes("/opt/axon/libaxon_pjrt.so")
        if hook is not None:
            mod.set_axon_ntff_profile_hook(hook)
    except Exception as e:
        print(f"ntff shim: hook install failed: {e}", file=sys.stderr)


def kernel(**inputs):
    import concourse.bass_utils as bass_utils
    from concourse.bass_utils import run_bass_kernel_spmd

    if "nc" not in _CACHE:
        _CACHE["nc"] = _build_nc()
    nc = _CACHE["nc"]

    in_maps = _prepare_core_inputs(inputs)
    trace = os.environ.get("TRNK_TRACE", "0") == "1"
    if trace:
        _install_ntff_shim()
        # no S3 in this container; keep artifacts local
        bass_utils.upload_artifacts = lambda d: d
    res = run_bass_kernel_spmd(nc, in_maps, core_ids=list(range(NCORES)),
                               trace=trace)
    _CACHE["last_results"] = res

    o_b = np.asarray(inputs["o_b"], np.float32)
    out = np.zeros((B, S, D), np.float32)
    for c in range(NCORES):
        b = c // GPC
        out[b] += res.results[c]["out"].reshape(S, D).astype(np.float32)
    out += o_b[None, None, :]
    return out


if __name__ == "__main__":
    # smoke test against random inputs (no reference available standalone)
    rng = np.random.default_rng(0)
    ins = {
        "query": rng.standard_normal((B, S, D)).astype(np.float32),
        "q_w": (rng.standard_normal((D, D)) * 0.03).astype(np.float32),
        "q_b": np.zeros(D, np.float32),
        "k_w": (rng.standard_normal((D, D)) * 0.03).astype(np.float32),
        "k_b": np.zeros(D, np.float32),
        "v_w": (rng.standard_normal((D, D)) * 0.03).astype(np.float32),
        "v_b": np.zeros(D, np.float32),
        "o_w": (rng.standard_normal((D, D)) * 0.03).astype(np.float32),
        "o_b": np.zeros(D, np.float32),
        "q_ln_g": np.ones(HD, np.float32),
        "q_ln_b": np.zeros(HD, np.float32),
        "k_ln_g": np.ones(HD, np.float32),
        "k_ln_b": np.zeros(HD, np.float32),
    }
    out = kernel(**ins)
    print("out", out.shape, out.dtype, float(np.abs(out).max()))


# revision 29
# speedup vs baseline: 1.0682x; 1.0682x over previous
"""Trainium2 Bass kernel: multi-head attention with per-head QK LayerNorm.

Problem shapes: B=2, S=2048, D=1024, H=16 heads, head_dim=64, fp32 in/out.

Sharding (8 cores): core c handles batch b = c//4 and head-group g = c%4
(4 heads = 256 qkv dims). Each core computes its heads' attention and a
partial out-projection; the host sums the 4 partials per batch entry
(tensor-parallel all-reduce done on host at unshard time) and adds o_b.

Key algebraic restructurings (all exact, modulo fp rounding):
  - LN mean subtraction and gain g are linear => folded into q_w/k_w (and
    biases) on the host.  Kernel computes qg = g*(q - mean(q)) directly.
  - LN variance = sum(w_d * qg_d^2) with w_d = 1/(64*g_d^2): computed on
    device from qg^2 via small block-diagonal stats matmuls.
  - rstd_q is folded into qT columns and tau*rstd_k into kT columns
    (via partition-broadcast DMAs), so softmax is a bare exp() of the
    raw scores.  Scores are computed TRANSPOSED: [kv on partitions,
    q on free], which feeds AV directly with no PE transposes.
  - softmax max-subtraction is skipped: post-LN rows have norm 8, so
    |scores| <= 8 and exp() stays in range.
  - sum(exp) over kv falls out of the AV matmul via a ones-column
    appended to V.  Normalization happens on attT eviction.

Perf notes (v2, fp16 + software-pipelined emission):
  - All matmul operands fp16, all matmuls N=512.  fp16 streams at the
    full 1 col/cycle PE rate and enables FWL weight loads; 11 mantissa
    bits keep final rel err ~1e-3 (bf16 would be marginal).
  - Phase 2 is ACT(exp)-bound (128 x 1147ns merged exps).  Engine
    queues execute in order, so emission is software-pipelined:
    QK(j+1) is emitted BEFORE exp(j)/AV(j) so the PE never sits behind
    an exp-dependent AV when the next scores could be computing.
  - QK pairs go to row tiles (0,0)/(64,0) (lhsT partitions 0-63/64-127)
    and run CONCURRENTLY on the PE (measured 109ns each @N=512 warm).
  - The c1 projection chains, v is upfront, out-projections and the
    remaining q chains are WOVEN into the exp stream as PE filler --
    this both hides phase-1 latency and keeps PE busy% high enough
    that the HAM clock gate stays at 2.4 GHz.
  - Projection chains are split A (8 proj mms + evict + square) /
    B (stats mm + sqrt + recip + bcast + scale) and B is emitted >=2
    exp-periods after A so the PE queue never stalls on GpSimd square.
  - PSUM: scores 2x[128,2,512] (4 banks) + AV accum 2 + acc pool
    (proj/stats/out-proj) 2 = 8 banks exactly.
"""

import os
import sys

import numpy as np

for _p in ("/opt/trn_rl_repo",):
    if _p not in sys.path:
        sys.path.append(_p)

# ---- problem constants (hardcoded; kernel.py must be self-contained) ----
B, S, D, H, HD = 2, 2048, 1024, 16, 64
EPS = 1e-5
NCORES = 8
GPC = 4            # cores per batch entry (head-groups)
HL = H // GPC      # 4 local heads
DL = HL * HD       # 256 local qkv dims
P = 128
KC = D // P        # 8 contraction chunks for projections
CL = DL // P       # 2 local-dim partition chunks (head pairs)
SB = 512           # free-dim block (= one PSUM bank of fp32)
NSB = S // SB      # 4 blocks
NKV = S // P       # 16 kv chunks
STW = 33           # stats lhsT cols: head vars at partitions 0 and 32

_CACHE = {}


def _build_nc():
    """Build the (single, SPMD-shared) Bass program for one core."""
    import concourse.bass as bass
    import concourse.mybir as mybir
    import concourse.tile as tile
    from concourse import bacc
    from concourse.dve_ops import RECIPROCAL_APPROX_FAST, RECIP_APPROX_FAST_CONSTS

    f32 = mybir.dt.float32
    f16 = mybir.dt.float16
    AF = mybir.ActivationFunctionType
    rc = RECIP_APPROX_FAST_CONSTS

    def recip(nc, out, in_):
        # ~51-ULP reciprocal in a single DVE pass (vs ~6 cyc/elem exact).
        return nc.vector._custom_dve(
            RECIPROCAL_APPROX_FAST, out=out, in0=in_,
            s0=rc["s0"], s1=rc["s1"], imm2=rc["imm2"],
        )

    nc = bacc.Bacc(trn_type="TRN2")

    xT_d = nc.dram_tensor("xT", [KC, P, S], f16, kind="ExternalInput")
    wqT_d = nc.dram_tensor("wqT", [KC, P, DL], f16, kind="ExternalInput")
    wkT_d = nc.dram_tensor("wkT", [KC, P, DL], f16, kind="ExternalInput")
    wvT_d = nc.dram_tensor("wvT", [KC, P, DL], f16, kind="ExternalInput")
    woT_d = nc.dram_tensor("woT", [CL, P, D], f16, kind="ExternalInput")
    qb_d = nc.dram_tensor("qb", [CL, P, 1], f32, kind="ExternalInput")
    kb_d = nc.dram_tensor("kb", [CL, P, 1], f32, kind="ExternalInput")
    vb_d = nc.dram_tensor("vb", [1, DL], f32, kind="ExternalInput")
    wsq_d = nc.dram_tensor("wsq", [CL, P, STW], f16, kind="ExternalInput")
    wsk_d = nc.dram_tensor("wsk", [CL, P, STW], f16, kind="ExternalInput")
    out_d = nc.dram_tensor("out", [NKV, P, D], f16, kind="ExternalOutput")

    with tile.TileContext(nc) as tc:
        with tc.tile_pool(name="big", bufs=1) as big:
            # ---- persistent SBUF; DMA issue order = need order ----
            xt = [big.tile([P, S], f16, name=f"xt{k}") for k in range(KC)]
            wk_sb = [big.tile([P, DL], f16, name=f"wk{k}") for k in range(KC)]
            wq_sb = [big.tile([P, DL], f16, name=f"wq{k}") for k in range(KC)]
            wv_sb = [big.tile([P, DL], f16, name=f"wv{k}") for k in range(KC)]
            # xt arrives in per-s-block quarters, sb0 first, so the first
            # projection chain starts after ~1MB instead of the full 4MB.
            for k in range(KC):
                nc.sync.dma_start(xt[k][:, 0:SB], xT_d[k, :, 0:SB])
                nc.sync.dma_start(wk_sb[k], wkT_d[k])
            kb_sb = big.tile([P, CL, 1], f32, name="kb_sb")
            qb_sb = big.tile([P, CL, 1], f32, name="qb_sb")
            wsq_sb = big.tile([P, CL, STW], f16, name="wsq_sb")
            wsk_sb = big.tile([P, CL, STW], f16, name="wsk_sb")
            for c in range(CL):
                nc.sync.dma_start(kb_sb[:, c, :], kb_d[c])
                nc.sync.dma_start(qb_sb[:, c, :], qb_d[c])
                nc.sync.dma_start(wsq_sb[:, c, :], wsq_d[c])
                nc.sync.dma_start(wsk_sb[:, c, :], wsk_d[c])
            for sb in range(1, NSB):
                for k in range(KC):
                    nc.sync.dma_start(xt[k][:, sb * SB:(sb + 1) * SB],
                                      xT_d[k, :, sb * SB:(sb + 1) * SB])
            for k in range(KC):
                nc.sync.dma_start(wq_sb[k], wqT_d[k])
            for k in range(KC):
                nc.sync.dma_start(wv_sb[k], wvT_d[k])
            vb_bc = big.tile([P, DL], f32, name="vb_bc")
            nc.sync.dma_start(vb_bc, vb_d[:].to_broadcast((P, DL)))
            wo_sb = big.tile([P, CL, D], f16, name="wo_sb")
            for c in range(CL):
                nc.sync.dma_start(wo_sb[:, c, :], woT_d[c])

            kT_sb = big.tile([P, CL, S], f16, name="kT_sb")
            qTs_sb = big.tile([P, CL, S], f16, name="qTs_sb")
            vaug_sb = big.tile([P, NKV, HL, HD + 1], f16, name="vaug_sb")
            attT_sb = big.tile([P, CL, S], f16, name="attT_sb")
            nc.vector.memset(vaug_sb[:, :, :, HD:HD + 1], 1.0)
            # onesel broadcasts rstd rows (partitions 0 and 32) to the 128
            # qkv partitions via a matmul: col m reads partition 0 (m<64)
            # or partition 32 (m>=64).
            onesel = big.tile([STW, P], f16, name="onesel")
            nc.vector.memset(onesel, 0.0)
            nc.vector.memset(onesel[0:1, 0:HD], 1.0)
            nc.vector.memset(onesel[32:33, HD:P], 1.0)

            with tc.tile_pool(name="acc", bufs=2, space="PSUM") as acc, \
                 tc.tile_pool(name="qk", bufs=2, space="PSUM") as qk, \
                 tc.tile_pool(name="av", bufs=1, space="PSUM") as avp, \
                 tc.tile_pool(name="sq", bufs=3) as sq, \
                 tc.tile_pool(name="ev", bufs=4) as ev, \
                 tc.tile_pool(name="ex", bufs=5) as exp_pool:

                SIDES = {
                    "k": (wk_sb, kb_sb, wsk_sb, kT_sb, 64.0),
                    "q": (wq_sb, qb_sb, wsq_sb, qTs_sb, 1.0),
                }
                i32 = mybir.dt.int32
                ALU = mybir.AluOpType
                RSQRT_MAGIC = 0x5F3759DF

                def dve_rsqrt(z, rr_out):
                    """rr_out(f16) = z**-0.5 via quake seed + 2 Newton
                    iterations, entirely on the Vector engine (no ACT table,
                    no broken partition-broadcast)."""
                    sh = ev.tile([STW, SB], i32, name="sh", bufs=2)
                    nc.vector.tensor_scalar(
                        sh, z.bitcast(i32), 1, None,
                        op0=ALU.logical_shift_right)
                    y0i = ev.tile([STW, SB], i32, name="y0i", bufs=2)
                    nc.vector.tensor_scalar(
                        y0i, sh, -1, RSQRT_MAGIC,
                        op0=ALU.mult, op1=ALU.add)
                    y = y0i.bitcast(f32)
                    for it in range(2):
                        t = ev.tile([STW, SB], f32, name="t", tag="t", bufs=4)
                        nc.vector.tensor_mul(t, z, y)
                        t2 = ev.tile([STW, SB], f32, name="t2", tag="t2",
                                     bufs=4)
                        nc.vector.tensor_mul(t2, t, y)
                        w = ev.tile([STW, SB], f32, name="w", tag="w", bufs=4)
                        nc.vector.tensor_scalar(
                            w, t2, -0.5, 1.5, op0=ALU.mult, op1=ALU.add)
                        if it == 0:
                            y1 = ev.tile([STW, SB], f32, name="y1", bufs=2)
                            nc.vector.tensor_mul(y1, y, w)
                            y = y1
                        else:
                            nc.vector.tensor_mul(rr_out, y, w)

                def chain_items(side, c, sb):
                    """q/k projection chain, split A/B1/B2 so the PE queue
                    never waits on the GpSimd square (A->B1) or the DVE
                    rsqrt latency (B1->B2)."""
                    wlist, bcol, wst, dst, scv = SIDES[side]
                    st = {}

                    def part_a():
                        ph = acc.tile([P, SB], f32, name="ph", tag="acc")
                        for k in range(KC):
                            nc.tensor.matmul(
                                ph, wlist[k][:, c * P:(c + 1) * P],
                                xt[k][:, sb * SB:(sb + 1) * SB],
                                start=(k == 0), stop=(k == KC - 1),
                            )
                        tr = sq.tile([P, SB], f16, name="tr_t")
                        nc.vector.tensor_scalar_add(tr, ph, bcol[:, c, :])
                        # DVE, not GpSimd: mixing tensor ops with
                        # partition_broadcast on GpSimd ping-pongs its ucode
                        # library (~3-6us hidden LIBRARY_RELOAD per switch).
                        qsq = sq.tile([P, SB], f16, name="sq_t")
                        nc.vector.tensor_mul(qsq, tr, tr)
                        st["tr"], st["qsq"] = tr, qsq

                    def part_b1():
                        # stats lhsT has 33 cols: head0 var -> partition 0,
                        # head1 var -> partition 32 (engines may only access
                        # partition bases aligned to 32).
                        stp = acc.tile([STW, SB], f32, name="stp", tag="acc")
                        nc.tensor.matmul(stp, wst[:, c, :], st["qsq"],
                                         start=True, stop=True)
                        z = ev.tile([STW, SB], f32, name="z", bufs=2)
                        nc.vector.tensor_scalar(
                            z, stp, scv, scv * EPS,
                            op0=ALU.mult, op1=ALU.add)
                        rr = ev.tile([STW, SB], f16, name="rr", bufs=2)
                        dve_rsqrt(z, rr)
                        st["rr"] = rr

                    def part_b2():
                        # broadcast rstd rows to all 128 partitions on the PE
                        # (partition_broadcast with out base 64 is broken on
                        # HW; SBUF->SBUF broadcast DMA has multi-us latency).
                        qsc = acc.tile([P, SB], f32, name="qsc", tag="acc")
                        nc.tensor.matmul(qsc, onesel, st["rr"],
                                         start=True, stop=True)
                        nc.vector.tensor_mul(
                            dst[:, c, sb * SB:(sb + 1) * SB], st["tr"], qsc)

                    return [("chain", part_a), ("chain", part_b1),
                            ("chain", part_b2)]

                def v_item(mc):
                    def f():
                        pv = acc.tile([P, SB], f32, name="pv",
                                      tag="acc")[:, :DL]
                        for k in range(KC):
                            nc.tensor.matmul(
                                pv, xt[k][:, mc * P:(mc + 1) * P], wv_sb[k],
                                start=(k == 0), stop=(k == KC - 1),
                            )
                        nc.vector.tensor_add(
                            vaug_sb[:, mc, :, 0:HD],
                            pv.rearrange("p (h d) -> p h d", d=HD),
                            vb_bc.rearrange("p (h d) -> p h d", d=HD),
                        )
                    return [("v", f)]

                def op_item(m, nb, use_qk=False):
                    def f():
                        if use_qk:
                            pon = qk.tile([P, 2, SB], f32,
                                          name="qk_t")[:, 0, :]
                        else:
                            pon = acc.tile([P, SB], f32, name="pon",
                                           tag="acc")
                        for c in range(CL):
                            nc.tensor.matmul(
                                pon, attT_sb[:, c, m * P:(m + 1) * P],
                                wo_sb[:, c, nb * SB:(nb + 1) * SB],
                                start=(c == 0), stop=(c == CL - 1),
                            )
                        osb = ev.tile([P, SB], f16, name="osb")
                        nc.vector.tensor_copy(osb, pon)
                        nc.sync.dma_start(
                            out_d[m, :, nb * SB:(nb + 1) * SB], osb)
                    return [("op", f)]

                # ---- upfront: k(c0) x4, q(c0,sb0), v0-v1 only; the rest
                # of v and all other chains weave into the exp stream.
                # A/B1/B2 staged so the PE never waits on the GpSimd square
                # (A->B1) or the DVE rsqrt (B1->B2).
                ch = [chain_items("k", 0, 0), chain_items("k", 0, 1),
                      chain_items("k", 0, 2), chain_items("k", 0, 3),
                      chain_items("q", 0, 0)]
                A = [c[0][1] for c in ch]
                B1 = [c[1][1] for c in ch]
                B2 = [c[2][1] for c in ch]
                for fn in (A[0], A[1], B1[0], A[2], B1[1], B2[0],
                           A[3], B1[2], B2[1], A[4], B1[3], B2[2],
                           B1[4], v_item(0)[0][1], B2[3],
                           v_item(1)[0][1], B2[4]):
                    fn()

                # ---- filler schedule: block idx -> list of (kind, fn) ----
                fillers = {i: [] for i in range(8)}
                PAD = ("pad", lambda: None)
                # block order is c0-major: blocks 0-3 = (qb0..3, c0),
                # blocks 4-7 = (qb0..3, c1).  Every chain's B2 must be
                # emitted before the first QK of the block that consumes its
                # kT/qTs slice (emission order IS the dependency order).
                vs = [v_item(mc)[0] for mc in range(2, NKV)]
                q01 = chain_items("q", 0, 1)
                # block0: v chunks just-in-time (v(mc) >=2 periods before its
                # AV) with q(c0,sb1) finishing by slot 14.
                fillers[0] = (vs[0:8] + [q01[0]] + vs[8:10] + [q01[1]] +
                              vs[10:12] + [q01[2]] + vs[12:14])

                def two_chains(ca, cb):
                    return [ca[0], cb[0], ca[1], cb[1], PAD, ca[2], cb[2]]

                fillers[1] = two_chains(chain_items("q", 0, 2),
                                        chain_items("k", 1, 0))
                q03, k11, q10 = (chain_items("q", 0, 3),
                                 chain_items("k", 1, 1),
                                 chain_items("q", 1, 0))
                fillers[2] = [q03[0], k11[0], q03[1], q10[0], k11[1], PAD,
                              q03[2], q10[1], k11[2], PAD, q10[2]]
                fillers[3] = two_chains(chain_items("k", 1, 2),
                                        chain_items("k", 1, 3))

                def one_chain(c):
                    return [c[0], PAD, c[1], PAD, PAD, c[2]]

                fillers[4] = one_chain(chain_items("q", 1, 1))
                fillers[5] = one_chain(chain_items("q", 1, 2))
                fillers[6] = one_chain(chain_items("q", 1, 3))
                # out-projections: op(qb) needs attT(qb,c0) [block qb] and
                # attT(qb,c1) [block 4+qb]
                opi = {qb: [op_item(m, nb)[0]
                            for m in range(qb * 4, qb * 4 + 4)
                            for nb in range(D // SB)]
                       for qb in range(NSB - 1)}
                fillers[5] += opi[0]
                fillers[6] += opi[1]
                fillers[7] = opi[2]
                tail_ops = [op_item(m, nb, use_qk=(m + nb) % 2 == 1)[0]
                            for m in range(12, 16)
                            for nb in range(D // SB)]

                # ---- phase 2: software-pipelined attention stream ----
                blocks = [(qb, c) for c in range(CL) for qb in range(NSB)]
                groups = [(bi, qb, c, j)
                          for bi, (qb, c) in enumerate(blocks)
                          for j in range(NKV)]
                sc_of = {}
                avs_of = {}

                def emit_qk(g):
                    bi, qb, c, j = groups[g]
                    sc2 = qk.tile([P, 2, SB], f32, name="qk_t")
                    q0 = qb * SB
                    for h in range(2):
                        po = h * HD
                        nc.tensor.matmul(
                            sc2[:, h, :],
                            kT_sb[po:po + HD, c, j * P:(j + 1) * P],
                            qTs_sb[po:po + HD, c, q0:q0 + SB],
                            start=True, stop=True,
                        )
                    sc_of[g] = sc2

                AVLAG = 4  # AV trails exp by 4 groups: absorbs the norm
                #            latency of the previous block (av bufs=1) without
                #            blocking the in-order PE queue / starving ACT.
                ex_of = {}

                def emit_exp(g):
                    sc2 = sc_of.pop(g)
                    ex2 = exp_pool.tile([P, 2, SB], f16, name="ex_t")
                    nc.scalar.activation(ex2, sc2, AF.Exp)
                    ex_of[g] = ex2

                def emit_av(g):
                    bi, qb, c, j = groups[g]
                    ex2 = ex_of.pop(g)
                    if j == 0:
                        avs_of[bi] = [
                            avp.tile([HD + 1, SB], f32, name=f"av{h}",
                                     tag=f"av{h}") for h in range(2)]
                    for h in range(2):
                        nc.tensor.matmul(
                            avs_of[bi][h],
                            vaug_sb[:, j, c * 2 + h, :],
                            ex2[:, h, :],
                            start=(j == 0), stop=(j == NKV - 1),
                        )
                    if j == NKV - 1:
                        avs = avs_of.pop(bi)
                        q0 = qb * SB
                        for h in range(2):
                            po = h * HD
                            # plain copy handles the partition shift (64->0);
                            # partition-shifted custom-DVE ops are not
                            # trustworthy on HW.
                            srow = ev.tile([1, SB], f32, name="srow")
                            nc.vector.tensor_copy(srow, avs[h][HD:HD + 1, :])
                            rrow = ev.tile([1, SB], f32, name="rrow")
                            recip(nc, rrow, srow)
                            rbc = ev.tile([HD, SB], f32, name="rbc")
                            nc.gpsimd.partition_broadcast(
                                rbc, rrow[0:1, :], HD)
                            nc.vector.tensor_mul(
                                attT_sb[po:po + HD, c, q0:q0 + SB],
                                avs[h][0:HD, :], rbc)

                emit_qk(0)
                NG = len(groups)
                for g in range(NG + AVLAG):
                    if g + 1 < NG:
                        emit_qk(g + 1)
                    if g < NG:
                        emit_exp(g)
                    if g - AVLAG >= 0:
                        emit_av(g - AVLAG)
                    if g < NG:
                        bi, qb, c, j = groups[g]
                        # filler items per kv chunk (ops only once attT of
                        # the previous qb has had time to normalize); pop 2
                        # when the remaining slots would not drain the list
                        fl = fillers[bi]
                        npop = 1
                        if len(fl) > NKV - j:
                            npop = 2
                        minj = {"op": 6, "chain": 3}
                        for _ in range(npop):
                            if fl and j >= minj.get(fl[0][0], 0):
                                fl.pop(0)[1]()

                # tail: out-projection of the last q-block
                for it in tail_ops:
                    it[1]()

    nc.compile()
    return nc


def _prepare_core_inputs(inputs):
    """Fold LN centering/gain into weights; shard per core; cast fp16."""
    q = np.asarray(inputs["query"], np.float32)
    q_w = np.asarray(inputs["q_w"], np.float64)
    k_w = np.asarray(inputs["k_w"], np.float64)
    v_w = np.asarray(inputs["v_w"], np.float32)
    o_w = np.asarray(inputs["o_w"], np.float32)
    q_b = np.asarray(inputs["q_b"], np.float64)
    k_b = np.asarray(inputs["k_b"], np.float64)
    v_b = np.asarray(inputs["v_b"], np.float32)
    q_g = np.asarray(inputs["q_ln_g"], np.float64)
    k_g = np.asarray(inputs["k_ln_g"], np.float64)

    def fold(w, b, g):
        # per head block (64 out-dims): center across the block, scale by g
        w = w.reshape(H, HD, D)
        w = (w - w.mean(axis=1, keepdims=True)) * g[None, :, None]
        b = b.reshape(H, HD)
        b = (b - b.mean(axis=1, keepdims=True)) * g[None, :]
        return w.reshape(D, D), b.reshape(D).astype(np.float32)

    wq_f, qb_f = fold(q_w, q_b, q_g)
    wk_f, kb_f = fold(k_w, k_b, k_g)

    def stat_w(g):
        # w_dd = 1/(64*g_d^2), laid out [CL, P, 33] block-diagonal per c-half
        # (head0 -> col 0, head1 -> col 32: partition-32-aligned outputs)
        w = np.zeros((CL, P, STW), np.float64)
        for c in range(CL):
            for h in range(2):
                w[c, h * HD:(h + 1) * HD, 32 * h] = 1.0 / (HD * g[:HD] ** 2)
        return w.astype(np.float16)

    wsq = stat_w(np.asarray(inputs["q_ln_g"], np.float64))
    wsk = stat_w(np.asarray(inputs["k_ln_g"], np.float64))

    in_maps = []
    for c in range(NCORES):
        b, g = divmod(c, GPC)
        rows = slice(g * DL, (g + 1) * DL)
        in_maps.append({
            "xT": np.ascontiguousarray(q[b].T).reshape(KC, P, S).astype(np.float16),
            "wqT": np.ascontiguousarray(wq_f[rows].T).reshape(KC, P, DL).astype(np.float16),
            "wkT": np.ascontiguousarray(wk_f[rows].T).reshape(KC, P, DL).astype(np.float16),
            "wvT": np.ascontiguousarray(v_w[rows].T).reshape(KC, P, DL).astype(np.float16),
            "woT": np.ascontiguousarray(o_w[:, rows].T).reshape(CL, P, D).astype(np.float16),
            "qb": np.ascontiguousarray(qb_f[rows]).reshape(CL, P, 1),
            "kb": np.ascontiguousarray(kb_f[rows]).reshape(CL, P, 1),
            "vb": np.ascontiguousarray(v_b[rows]).reshape(1, DL),
            "wsq": wsq,
            "wsk": wsk,
        })
    return in_maps


def _install_ntff_shim():
    """The agent image's antenv lacks axon_hooks; recreate it so
    run_bass_kernel_spmd(trace=True) can capture NTFF profiles."""
    import types

    try:
        import antenv.axon_hooks  # noqa: F401
        return
    except ImportError:
        pass
    import antenv
    mod = types.ModuleType("antenv.axon_hooks")
    mod._hook = None
    mod.set_axon_ntff_profile_hook = lambda h: setattr(mod, "_hook", h)
    mod.get_axon_ntff_profile_hook = lambda: mod._hook
    sys.modules["antenv.axon_hooks"] = mod
    antenv.axon_hooks = mod
    try:
        from trn_agent_boot.trn_boot import _ntff_profile_via_ctypes
        hook = _ntff_profile_via_ctypes("/opt/axon/libaxon_pjrt.so")
        if hook is not None:
            mod.set_axon_ntff_profile_hook(hook)
    except Exception as e:
        print(f"ntff shim: hook install failed: {e}", file=sys.stderr)


def kernel(**inputs):
    import concourse.bass_utils as bass_utils
    from concourse.bass_utils import run_bass_kernel_spmd

    if "nc" not in _CACHE:
        _CACHE["nc"] = _build_nc()
    nc = _CACHE["nc"]

    in_maps = _prepare_core_inputs(inputs)
    trace = os.environ.get("TRNK_TRACE", "0") == "1"
    if trace:
        _install_ntff_shim()
        # no S3 in this container; keep artifacts local
        bass_utils.upload_artifacts = lambda d: d
    res = run_bass_kernel_spmd(nc, in_maps, core_ids=list(range(NCORES)),
                               trace=trace)
    _CACHE["last_results"] = res

    o_b = np.asarray(inputs["o_b"], np.float32)
    out = np.zeros((B, S, D), np.float32)
    for c in range(NCORES):
        b = c // GPC
        out[b] += res.results[c]["out"].reshape(S, D).astype(np.float32)
    out += o_b[None, None, :]
    return out


if __name__ == "__main__":
    # smoke test against random inputs (no reference available standalone)
    rng = np.random.default_rng(0)
    ins = {
        "query": rng.standard_normal((B, S, D)).astype(np.float32),
        "q_w": (rng.standard_normal((D, D)) * 0.03).astype(np.float32),
        "q_b": np.zeros(D, np.float32),
        "k_w": (rng.standard_normal((D, D)) * 0.03).astype(np.float32),
        "k_b": np.zeros(D, np.float32),
        "v_w": (rng.standard_normal((D, D)) * 0.03).astype(np.float32),
        "v_b": np.zeros(D, np.float32),
        "o_w": (rng.standard_normal((D, D)) * 0.03).astype(np.float32),
        "o_b": np.zeros(D, np.float32),
        "q_ln_g": np.ones(HD, np.float32),
        "q_ln_b": np.zeros(HD, np.float32),
        "k_ln_g": np.ones(HD, np.float32),
        "k_ln_b": np.zeros(HD, np.float32),
    }
    out = kernel(**ins)
    print("out", out.shape, out.dtype, float(np.abs(out).max()))


# revision 30
# speedup vs baseline: 1.0765x; 1.0079x over previous
"""Trainium2 Bass kernel: multi-head attention with per-head QK LayerNorm.

Problem shapes: B=2, S=2048, D=1024, H=16 heads, head_dim=64, fp32 in/out.

Sharding (8 cores): core c handles batch b = c//4 and head-group g = c%4
(4 heads = 256 qkv dims). Each core computes its heads' attention and a
partial out-projection; the host sums the 4 partials per batch entry
(tensor-parallel all-reduce done on host at unshard time) and adds o_b.

Key algebraic restructurings (all exact, modulo fp rounding):
  - LN mean subtraction and gain g are linear => folded into q_w/k_w (and
    biases) on the host.  Kernel computes qg = g*(q - mean(q)) directly.
  - LN variance = sum(w_d * qg_d^2) with w_d = 1/(64*g_d^2): computed on
    device from qg^2 via small block-diagonal stats matmuls.
  - rstd_q is folded into qT columns and tau*rstd_k into kT columns
    (via partition-broadcast DMAs), so softmax is a bare exp() of the
    raw scores.  Scores are computed TRANSPOSED: [kv on partitions,
    q on free], which feeds AV directly with no PE transposes.
  - softmax max-subtraction is skipped: post-LN rows have norm 8, so
    |scores| <= 8 and exp() stays in range.
  - sum(exp) over kv falls out of the AV matmul via a ones-column
    appended to V.  Normalization happens on attT eviction.

Perf notes (v2, fp16 + software-pipelined emission):
  - All matmul operands fp16, all matmuls N=512.  fp16 streams at the
    full 1 col/cycle PE rate and enables FWL weight loads; 11 mantissa
    bits keep final rel err ~1e-3 (bf16 would be marginal).
  - Phase 2 is ACT(exp)-bound (128 x 1147ns merged exps).  Engine
    queues execute in order, so emission is software-pipelined:
    QK(j+1) is emitted BEFORE exp(j)/AV(j) so the PE never sits behind
    an exp-dependent AV when the next scores could be computing.
  - QK pairs go to row tiles (0,0)/(64,0) (lhsT partitions 0-63/64-127)
    and run CONCURRENTLY on the PE (measured 109ns each @N=512 warm).
  - The c1 projection chains, v is upfront, out-projections and the
    remaining q chains are WOVEN into the exp stream as PE filler --
    this both hides phase-1 latency and keeps PE busy% high enough
    that the HAM clock gate stays at 2.4 GHz.
  - Projection chains are split A (8 proj mms + evict + square) /
    B (stats mm + sqrt + recip + bcast + scale) and B is emitted >=2
    exp-periods after A so the PE queue never stalls on GpSimd square.
  - PSUM: scores 2x[128,2,512] (4 banks) + AV accum 2 + acc pool
    (proj/stats/out-proj) 2 = 8 banks exactly.
"""

import os
import sys

import numpy as np

for _p in ("/opt/trn_rl_repo",):
    if _p not in sys.path:
        sys.path.append(_p)

# ---- problem constants (hardcoded; kernel.py must be self-contained) ----
B, S, D, H, HD = 2, 2048, 1024, 16, 64
EPS = 1e-5
NCORES = 8
GPC = 4            # cores per batch entry (head-groups)
HL = H // GPC      # 4 local heads
DL = HL * HD       # 256 local qkv dims
P = 128
KC = D // P        # 8 contraction chunks for projections
CL = DL // P       # 2 local-dim partition chunks (head pairs)
SB = 512           # free-dim block (= one PSUM bank of fp32)
NSB = S // SB      # 4 blocks
NKV = S // P       # 16 kv chunks
STW = 33           # stats lhsT cols: head vars at partitions 0 and 32

_CACHE = {}


def _build_nc():
    """Build the (single, SPMD-shared) Bass program for one core."""
    import concourse.bass as bass
    import concourse.mybir as mybir
    import concourse.tile as tile
    from concourse import bacc
    from concourse.dve_ops import RECIPROCAL_APPROX_FAST, RECIP_APPROX_FAST_CONSTS

    f32 = mybir.dt.float32
    f16 = mybir.dt.float16
    AF = mybir.ActivationFunctionType
    rc = RECIP_APPROX_FAST_CONSTS

    def recip(nc, out, in_):
        # ~51-ULP reciprocal in a single DVE pass (vs ~6 cyc/elem exact).
        return nc.vector._custom_dve(
            RECIPROCAL_APPROX_FAST, out=out, in0=in_,
            s0=rc["s0"], s1=rc["s1"], imm2=rc["imm2"],
        )

    nc = bacc.Bacc(trn_type="TRN2")

    xT_d = nc.dram_tensor("xT", [KC, P, S], f16, kind="ExternalInput")
    wqT_d = nc.dram_tensor("wqT", [KC, P, DL], f16, kind="ExternalInput")
    wkT_d = nc.dram_tensor("wkT", [KC, P, DL], f16, kind="ExternalInput")
    wvT_d = nc.dram_tensor("wvT", [KC, P, DL], f16, kind="ExternalInput")
    woT_d = nc.dram_tensor("woT", [CL, P, D], f16, kind="ExternalInput")
    qb_d = nc.dram_tensor("qb", [CL, P, 1], f32, kind="ExternalInput")
    kb_d = nc.dram_tensor("kb", [CL, P, 1], f32, kind="ExternalInput")
    vb_d = nc.dram_tensor("vb", [1, DL], f32, kind="ExternalInput")
    wsq_d = nc.dram_tensor("wsq", [CL, P, STW], f16, kind="ExternalInput")
    wsk_d = nc.dram_tensor("wsk", [CL, P, STW], f16, kind="ExternalInput")
    out_d = nc.dram_tensor("out", [NKV, P, D], f16, kind="ExternalOutput")

    with tile.TileContext(nc) as tc:
        with tc.tile_pool(name="big", bufs=1) as big:
            # ---- persistent SBUF; DMA issue order = need order ----
            xt = [big.tile([P, S], f16, name=f"xt{k}") for k in range(KC)]
            wk_sb = [big.tile([P, DL], f16, name=f"wk{k}") for k in range(KC)]
            wq_sb = [big.tile([P, DL], f16, name=f"wq{k}") for k in range(KC)]
            wv_sb = [big.tile([P, DL], f16, name=f"wv{k}") for k in range(KC)]
            # xt arrives in per-s-block quarters, sb0 first, so the first
            # projection chain starts after ~1MB instead of the full 4MB.
            for k in range(KC):
                nc.sync.dma_start(xt[k][:, 0:SB], xT_d[k, :, 0:SB])
                nc.sync.dma_start(wk_sb[k], wkT_d[k])
            kb_sb = big.tile([P, CL, 1], f32, name="kb_sb")
            qb_sb = big.tile([P, CL, 1], f32, name="qb_sb")
            wsq_sb = big.tile([P, CL, STW], f16, name="wsq_sb")
            wsk_sb = big.tile([P, CL, STW], f16, name="wsk_sb")
            for c in range(CL):
                nc.sync.dma_start(kb_sb[:, c, :], kb_d[c])
                nc.sync.dma_start(qb_sb[:, c, :], qb_d[c])
                nc.sync.dma_start(wsq_sb[:, c, :], wsq_d[c])
                nc.sync.dma_start(wsk_sb[:, c, :], wsk_d[c])
            for sb in range(1, NSB):
                for k in range(KC):
                    nc.sync.dma_start(xt[k][:, sb * SB:(sb + 1) * SB],
                                      xT_d[k, :, sb * SB:(sb + 1) * SB])
            for k in range(KC):
                nc.sync.dma_start(wq_sb[k], wqT_d[k])
            for k in range(KC):
                nc.sync.dma_start(wv_sb[k], wvT_d[k])
            vb_bc = big.tile([P, DL], f32, name="vb_bc")
            nc.sync.dma_start(vb_bc, vb_d[:].to_broadcast((P, DL)))
            wo_sb = big.tile([P, CL, D], f16, name="wo_sb")
            for c in range(CL):
                nc.sync.dma_start(wo_sb[:, c, :], woT_d[c])

            kT_sb = big.tile([P, CL, S], f16, name="kT_sb")
            qTs_sb = big.tile([P, CL, S], f16, name="qTs_sb")
            vaug_sb = big.tile([P, NKV, HL, HD + 1], f16, name="vaug_sb")
            attT_sb = big.tile([P, CL, S], f16, name="attT_sb")
            nc.vector.memset(vaug_sb[:, :, :, HD:HD + 1], 1.0)
            # onesel broadcasts rstd rows (partitions 0 and 32) to the 128
            # qkv partitions via a matmul: col m reads partition 0 (m<64)
            # or partition 32 (m>=64).
            onesel = big.tile([STW, P], f16, name="onesel")
            nc.vector.memset(onesel, 0.0)
            nc.vector.memset(onesel[0:1, 0:HD], 1.0)
            nc.vector.memset(onesel[32:33, HD:P], 1.0)

            with tc.tile_pool(name="acc", bufs=2, space="PSUM") as acc, \
                 tc.tile_pool(name="qk", bufs=2, space="PSUM") as qk, \
                 tc.tile_pool(name="av", bufs=1, space="PSUM") as avp, \
                 tc.tile_pool(name="sq", bufs=5) as sq, \
                 tc.tile_pool(name="ev", bufs=4) as ev, \
                 tc.tile_pool(name="ex", bufs=5) as exp_pool:

                SIDES = {
                    "k": (wk_sb, kb_sb, wsk_sb, kT_sb, 64.0),
                    "q": (wq_sb, qb_sb, wsq_sb, qTs_sb, 1.0),
                }
                i32 = mybir.dt.int32
                ALU = mybir.AluOpType
                RSQRT_MAGIC = 0x5F3759DF

                def dve_rsqrt(z, rr_out):
                    """rr_out(f16) = z**-0.5 via quake seed + 2 Newton
                    iterations, entirely on the Vector engine (no ACT table,
                    no broken partition-broadcast)."""
                    sh = ev.tile([STW, SB], i32, name="sh", bufs=3)
                    nc.vector.tensor_scalar(
                        sh, z.bitcast(i32), 1, None,
                        op0=ALU.logical_shift_right)
                    y0i = ev.tile([STW, SB], i32, name="y0i", bufs=3)
                    nc.vector.tensor_scalar(
                        y0i, sh, -1, RSQRT_MAGIC,
                        op0=ALU.mult, op1=ALU.add)
                    y = y0i.bitcast(f32)
                    for it in range(2):
                        t = ev.tile([STW, SB], f32, name="t", tag="t", bufs=6)
                        nc.vector.tensor_mul(t, z, y)
                        t2 = ev.tile([STW, SB], f32, name="t2", tag="t2",
                                     bufs=6)
                        nc.vector.tensor_mul(t2, t, y)
                        w = ev.tile([STW, SB], f32, name="w", tag="w", bufs=6)
                        nc.vector.tensor_scalar(
                            w, t2, -0.5, 1.5, op0=ALU.mult, op1=ALU.add)
                        if it == 0:
                            y1 = ev.tile([STW, SB], f32, name="y1", bufs=3)
                            nc.vector.tensor_mul(y1, y, w)
                            y = y1
                        else:
                            nc.vector.tensor_mul(rr_out, y, w)

                def chain_items(side, c, sb):
                    """q/k projection chain, split A/B1/B2 so the PE queue
                    never waits on the GpSimd square (A->B1) or the DVE
                    rsqrt latency (B1->B2)."""
                    wlist, bcol, wst, dst, scv = SIDES[side]
                    st = {}

                    def part_a():
                        ph = acc.tile([P, SB], f32, name="ph", tag="acc")
                        for k in range(KC):
                            nc.tensor.matmul(
                                ph, wlist[k][:, c * P:(c + 1) * P],
                                xt[k][:, sb * SB:(sb + 1) * SB],
                                start=(k == 0), stop=(k == KC - 1),
                            )
                        tr = sq.tile([P, SB], f16, name="tr_t")
                        nc.vector.tensor_scalar_add(tr, ph, bcol[:, c, :])
                        # DVE, not GpSimd: mixing tensor ops with
                        # partition_broadcast on GpSimd ping-pongs its ucode
                        # library (~3-6us hidden LIBRARY_RELOAD per switch).
                        qsq = sq.tile([P, SB], f16, name="sq_t")
                        nc.vector.tensor_mul(qsq, tr, tr)
                        st["tr"], st["qsq"] = tr, qsq

                    def part_b1():
                        # stats lhsT has 33 cols: head0 var -> partition 0,
                        # head1 var -> partition 32 (engines may only access
                        # partition bases aligned to 32).
                        stp = acc.tile([STW, SB], f32, name="stp", tag="acc")
                        nc.tensor.matmul(stp, wst[:, c, :], st["qsq"],
                                         start=True, stop=True)
                        z = ev.tile([STW, SB], f32, name="z", bufs=3)
                        nc.vector.tensor_scalar(
                            z, stp, scv, scv * EPS,
                            op0=ALU.mult, op1=ALU.add)
                        rr = ev.tile([STW, SB], f16, name="rr", bufs=3)
                        dve_rsqrt(z, rr)
                        st["rr"] = rr

                    def part_b2():
                        # broadcast rstd rows to all 128 partitions on the PE
                        # (partition_broadcast with out base 64 is broken on
                        # HW; SBUF->SBUF broadcast DMA has multi-us latency).
                        qsc = acc.tile([P, SB], f32, name="qsc", tag="acc")
                        nc.tensor.matmul(qsc, onesel, st["rr"],
                                         start=True, stop=True)
                        nc.vector.tensor_mul(
                            dst[:, c, sb * SB:(sb + 1) * SB], st["tr"], qsc)

                    return [("chain", part_a), ("chain", part_b1),
                            ("chain", part_b2)]

                def v_item(mc):
                    def f():
                        pv = acc.tile([P, SB], f32, name="pv",
                                      tag="acc")[:, :DL]
                        for k in range(KC):
                            nc.tensor.matmul(
                                pv, xt[k][:, mc * P:(mc + 1) * P], wv_sb[k],
                                start=(k == 0), stop=(k == KC - 1),
                            )
                        nc.vector.tensor_add(
                            vaug_sb[:, mc, :, 0:HD],
                            pv.rearrange("p (h d) -> p h d", d=HD),
                            vb_bc.rearrange("p (h d) -> p h d", d=HD),
                        )
                    return [("v", f)]

                def op_item(m, nb, use_qk=False):
                    def f():
                        if use_qk:
                            pon = qk.tile([P, 2, SB], f32,
                                          name="qk_t")[:, 0, :]
                        else:
                            pon = acc.tile([P, SB], f32, name="pon",
                                           tag="acc")
                        for c in range(CL):
                            nc.tensor.matmul(
                                pon, attT_sb[:, c, m * P:(m + 1) * P],
                                wo_sb[:, c, nb * SB:(nb + 1) * SB],
                                start=(c == 0), stop=(c == CL - 1),
                            )
                        osb = ev.tile([P, SB], f16, name="osb")
                        nc.vector.tensor_copy(osb, pon)
                        nc.sync.dma_start(
                            out_d[m, :, nb * SB:(nb + 1) * SB], osb)
                    return [("op", f)]

                # ---- upfront: k(c0) x4, q(c0,sb0), v0-v1 only; the rest
                # of v and all other chains weave into the exp stream.
                # A/B1/B2 staged so the PE never waits on the GpSimd square
                # (A->B1) or the DVE rsqrt (B1->B2).
                ch = [chain_items("k", 0, 0), chain_items("k", 0, 1),
                      chain_items("k", 0, 2), chain_items("k", 0, 3),
                      chain_items("q", 0, 0)]
                A = [c[0][1] for c in ch]
                B1 = [c[1][1] for c in ch]
                B2 = [c[2][1] for c in ch]
                for fn in (A[0], A[1], B1[0], A[2], B1[1], B2[0],
                           A[3], B1[2], B2[1], A[4], B1[3], B2[2],
                           B1[4], v_item(0)[0][1], B2[3],
                           v_item(1)[0][1], B2[4]):
                    fn()

                # ---- filler schedule: block idx -> list of (kind, fn) ----
                fillers = {i: [] for i in range(8)}
                PAD = ("pad", lambda: None)
                # block order is c0-major: blocks 0-3 = (qb0..3, c0),
                # blocks 4-7 = (qb0..3, c1).  Every chain's B2 must be
                # emitted before the first QK of the block that consumes its
                # kT/qTs slice (emission order IS the dependency order).
                vs = [v_item(mc)[0] for mc in range(2, NKV)]
                q01 = chain_items("q", 0, 1)
                # block0: v chunks just-in-time (v(mc) >=2 periods before its
                # AV) with q(c0,sb1) finishing by slot 14.
                fillers[0] = (vs[0:8] + [q01[0]] + vs[8:10] + [q01[1]] +
                              vs[10:12] + [q01[2]] + vs[12:14])

                def two_chains(ca, cb):
                    return [ca[0], cb[0], ca[1], cb[1], PAD, ca[2], cb[2]]

                fillers[1] = two_chains(chain_items("q", 0, 2),
                                        chain_items("k", 1, 0))
                q03, k11, q10 = (chain_items("q", 0, 3),
                                 chain_items("k", 1, 1),
                                 chain_items("q", 1, 0))
                fillers[2] = [q03[0], k11[0], q03[1], q10[0], k11[1], PAD,
                              q03[2], q10[1], k11[2], PAD, q10[2]]
                fillers[3] = two_chains(chain_items("k", 1, 2),
                                        chain_items("k", 1, 3))

                def one_chain(c):
                    return [c[0], PAD, c[1], PAD, PAD, c[2]]

                fillers[4] = one_chain(chain_items("q", 1, 1))
                fillers[5] = one_chain(chain_items("q", 1, 2))
                fillers[6] = one_chain(chain_items("q", 1, 3))
                # out-projections: op(qb) needs attT(qb,c0) [block qb] and
                # attT(qb,c1) [block 4+qb]
                opi = {qb: [op_item(m, nb)[0]
                            for m in range(qb * 4, qb * 4 + 4)
                            for nb in range(D // SB)]
                       for qb in range(NSB - 1)}
                fillers[5] += opi[0]
                fillers[6] += opi[1]
                fillers[7] = opi[2]
                tail_ops = [op_item(m, nb, use_qk=(m + nb) % 2 == 1)[0]
                            for m in range(12, 16)
                            for nb in range(D // SB)]

                # ---- phase 2: software-pipelined attention stream ----
                blocks = [(qb, c) for c in range(CL) for qb in range(NSB)]
                groups = [(bi, qb, c, j)
                          for bi, (qb, c) in enumerate(blocks)
                          for j in range(NKV)]
                sc_of = {}
                avs_of = {}

                def emit_qk(g):
                    bi, qb, c, j = groups[g]
                    sc2 = qk.tile([P, 2, SB], f32, name="qk_t")
                    q0 = qb * SB
                    for h in range(2):
                        po = h * HD
                        nc.tensor.matmul(
                            sc2[:, h, :],
                            kT_sb[po:po + HD, c, j * P:(j + 1) * P],
                            qTs_sb[po:po + HD, c, q0:q0 + SB],
                            start=True, stop=True,
                        )
                    sc_of[g] = sc2

                AVLAG = 4  # AV trails exp by 4 groups: absorbs the norm
                #            latency of the previous block (av bufs=1) without
                #            blocking the in-order PE queue / starving ACT.
                ex_of = {}

                def emit_exp(g):
                    sc2 = sc_of.pop(g)
                    ex2 = exp_pool.tile([P, 2, SB], f16, name="ex_t")
                    nc.scalar.activation(ex2, sc2, AF.Exp)
                    ex_of[g] = ex2

                def emit_av(g):
                    bi, qb, c, j = groups[g]
                    ex2 = ex_of.pop(g)
                    if j == 0:
                        avs_of[bi] = [
                            avp.tile([HD + 1, SB], f32, name=f"av{h}",
                                     tag=f"av{h}") for h in range(2)]
                    for h in range(2):
                        nc.tensor.matmul(
                            avs_of[bi][h],
                            vaug_sb[:, j, c * 2 + h, :],
                            ex2[:, h, :],
                            start=(j == 0), stop=(j == NKV - 1),
                        )
                    if j == NKV - 1:
                        avs = avs_of.pop(bi)
                        q0 = qb * SB
                        for h in range(2):
                            po = h * HD
                            # plain copy handles the partition shift (64->0);
                            # partition-shifted custom-DVE ops are not
                            # trustworthy on HW.
                            srow = ev.tile([1, SB], f32, name="srow")
                            nc.vector.tensor_copy(srow, avs[h][HD:HD + 1, :])
                            rrow = ev.tile([1, SB], f32, name="rrow")
                            recip(nc, rrow, srow)
                            rbc = ev.tile([HD, SB], f32, name="rbc")
                            nc.gpsimd.partition_broadcast(
                                rbc, rrow[0:1, :], HD)
                            nc.vector.tensor_mul(
                                attT_sb[po:po + HD, c, q0:q0 + SB],
                                avs[h][0:HD, :], rbc)

                emit_qk(0)
                NG = len(groups)
                for g in range(NG + AVLAG):
                    if g + 1 < NG:
                        emit_qk(g + 1)
                    if g < NG:
                        emit_exp(g)
                    if g - AVLAG >= 0:
                        emit_av(g - AVLAG)
                    if g < NG:
                        bi, qb, c, j = groups[g]
                        # filler items per kv chunk (ops only once attT of
                        # the previous qb has had time to normalize); pop 2
                        # when the remaining slots would not drain the list
                        fl = fillers[bi]
                        npop = 1
                        if len(fl) > NKV - j:
                            npop = 2
                        minj = {"op": 6, "chain": 3}
                        for _ in range(npop):
                            if fl and j >= minj.get(fl[0][0], 0):
                                fl.pop(0)[1]()

                # tail: out-projection of the last q-block
                for it in tail_ops:
                    it[1]()

    nc.compile()
    return nc


def _prepare_core_inputs(inputs):
    """Fold LN centering/gain into weights; shard per core; cast fp16."""
    q = np.asarray(inputs["query"], np.float32)
    q_w = np.asarray(inputs["q_w"], np.float64)
    k_w = np.asarray(inputs["k_w"], np.float64)
    v_w = np.asarray(inputs["v_w"], np.float32)
    o_w = np.asarray(inputs["o_w"], np.float32)
    q_b = np.asarray(inputs["q_b"], np.float64)
    k_b = np.asarray(inputs["k_b"], np.float64)
    v_b = np.asarray(inputs["v_b"], np.float32)
    q_g = np.asarray(inputs["q_ln_g"], np.float64)
    k_g = np.asarray(inputs["k_ln_g"], np.float64)

    def fold(w, b, g):
        # per head block (64 out-dims): center across the block, scale by g
        w = w.reshape(H, HD, D)
        w = (w - w.mean(axis=1, keepdims=True)) * g[None, :, None]
        b = b.reshape(H, HD)
        b = (b - b.mean(axis=1, keepdims=True)) * g[None, :]
        return w.reshape(D, D), b.reshape(D).astype(np.float32)

    wq_f, qb_f = fold(q_w, q_b, q_g)
    wk_f, kb_f = fold(k_w, k_b, k_g)

    def stat_w(g):
        # w_dd = 1/(64*g_d^2), laid out [CL, P, 33] block-diagonal per c-half
        # (head0 -> col 0, head1 -> col 32: partition-32-aligned outputs)
        w = np.zeros((CL, P, STW), np.float64)
        for c in range(CL):
            for h in range(2):
                w[c, h * HD:(h + 1) * HD, 32 * h] = 1.0 / (HD * g[:HD] ** 2)
        return w.astype(np.float16)

    wsq = stat_w(np.asarray(inputs["q_ln_g"], np.float64))
    wsk = stat_w(np.asarray(inputs["k_ln_g"], np.float64))

    in_maps = []
    for c in range(NCORES):
        b, g = divmod(c, GPC)
        rows = slice(g * DL, (g + 1) * DL)
        in_maps.append({
            "xT": np.ascontiguousarray(q[b].T).reshape(KC, P, S).astype(np.float16),
            "wqT": np.ascontiguousarray(wq_f[rows].T).reshape(KC, P, DL).astype(np.float16),
            "wkT": np.ascontiguousarray(wk_f[rows].T).reshape(KC, P, DL).astype(np.float16),
            "wvT": np.ascontiguousarray(v_w[rows].T).reshape(KC, P, DL).astype(np.float16),
            "woT": np.ascontiguousarray(o_w[:, rows].T).reshape(CL, P, D).astype(np.float16),
            "qb": np.ascontiguousarray(qb_f[rows]).reshape(CL, P, 1),
            "kb": np.ascontiguousarray(kb_f[rows]).reshape(CL, P, 1),
            "vb": np.ascontiguousarray(v_b[rows]).reshape(1, DL),
            "wsq": wsq,
            "wsk": wsk,
        })
    return in_maps


def _install_ntff_shim():
    """The agent image's antenv lacks axon_hooks; recreate it so
    run_bass_kernel_spmd(trace=True) can capture NTFF profiles."""
    import types

    try:
        import antenv.axon_hooks  # noqa: F401
        return
    except ImportError:
        pass
    import antenv
    mod = types.ModuleType("antenv.axon_hooks")
    mod._hook = None
    mod.set_axon_ntff_profile_hook = lambda h: setattr(mod, "_hook", h)
    mod.get_axon_ntff_profile_hook = lambda: mod._hook
    sys.modules["antenv.axon_hooks"] = mod
    antenv.axon_hooks = mod
    try:
        from trn_agent_boot.trn_boot import _ntff_profile_via_ctypes
        hook = _ntff_profile_via_ctypes("/opt/axon/libaxon_pjrt.so")
        if hook is not None:
            mod.set_axon_ntff_profile_hook(hook)
    except Exception as e:
        print(f"ntff shim: hook install failed: {e}", file=sys.stderr)


def kernel(**inputs):
    import concourse.bass_utils as bass_utils
    from concourse.bass_utils import run_bass_kernel_spmd

    if "nc" not in _CACHE:
        _CACHE["nc"] = _build_nc()
    nc = _CACHE["nc"]

    in_maps = _prepare_core_inputs(inputs)
    trace = os.environ.get("TRNK_TRACE", "0") == "1"
    if trace:
        _install_ntff_shim()
        # no S3 in this container; keep artifacts local
        bass_utils.upload_artifacts = lambda d: d
    res = run_bass_kernel_spmd(nc, in_maps, core_ids=list(range(NCORES)),
                               trace=trace)
    _CACHE["last_results"] = res

    o_b = np.asarray(inputs["o_b"], np.float32)
    out = np.zeros((B, S, D), np.float32)
    for c in range(NCORES):
        b = c // GPC
        out[b] += res.results[c]["out"].reshape(S, D).astype(np.float32)
    out += o_b[None, None, :]
    return out


if __name__ == "__main__":
    # smoke test against random inputs (no reference available standalone)
    rng = np.random.default_rng(0)
    ins = {
        "query": rng.standard_normal((B, S, D)).astype(np.float32),
        "q_w": (rng.standard_normal((D, D)) * 0.03).astype(np.float32),
        "q_b": np.zeros(D, np.float32),
        "k_w": (rng.standard_normal((D, D)) * 0.03).astype(np.float32),
        "k_b": np.zeros(D, np.float32),
        "v_w": (rng.standard_normal((D, D)) * 0.03).astype(np.float32),
        "v_b": np.zeros(D, np.float32),
        "o_w": (rng.standard_normal((D, D)) * 0.03).astype(np.float32),
        "o_b": np.zeros(D, np.float32),
        "q_ln_g": np.ones(HD, np.float32),
        "q_ln_b": np.zeros(HD, np.float32),
        "k_ln_g": np.ones(HD, np.float32),
        "k_ln_b": np.zeros(HD, np.float32),
    }
    out = kernel(**ins)
    print("out", out.shape, out.dtype, float(np.abs(out).max()))
